# revision 12
# baseline (speedup 1.0000x reference)
"""AWQ int4 dequant + matmul (M=4096, K=4096, N=11008) on 8 TRN2 NeuronCores.

Column-parallel: qweight/scales/qzeros/bias sharded along N (1376 per core),
x replicated. Per core: dequantize W' = q * s to bf16 on-chip (resident in
SBUF), x host-cast to bf16 and DMA-transposed straight from DRAM, bf16
matmuls with fp32 PSUM accumulation, add bias, write the output shard.

The zero-point term is algebraically hoisted out of the dequant:
  y = x @ ((q - z) * s) = x @ (q * s) - xg @ (z * s),  xg[m,g] = sum_k-in-g x[m,k]
xg (group sums) is computed on the host and the rank-32 correction runs as a
33rd k-chunk on the PE (K=32 matmul, row-packed via tile_position in phase A).
This removes both the zero-point subtract (DVE) and the z*s partition
broadcast (DMA), which otherwise starve the PE during the dequant phase.

Phase A is chunk-major over 8 concurrent PSUM groups (4 m-tiles x 2 n-tiles)
so the PE gets 4096 cycles of work per dequantized chunk instead of
head-of-line blocking on a single k-accumulation chain.
"""

import sys

if "/opt/trn_rl_repo" not in sys.path:
    sys.path.insert(0, "/opt/trn_rl_repo")

import ml_dtypes
import numpy as np

import concourse.mybir as mybir
import concourse.tile as tile
from concourse import bacc, bass_utils

# Problem shapes (hardcoded per contract)
M = 4096
K = 4096
N = 11008
G = 128  # AWQ group size
N_CORES = 8
NS = N // N_CORES  # 1376 output columns per core
CS = NS // 8  # 172 packed int32 columns per core
NCH = K // 128  # 32 k-chunks (each exactly one AWQ group)
N_TILES = [(0, 512), (512, 512), (1024, 352)]
PHA_M = 3  # m-tiles co-resident in phase A (x2 n-tiles = 6 PSUM banks)

BF16 = mybir.dt.bfloat16
F32 = mybir.dt.float32
I32 = mybir.dt.int32
U16 = mybir.dt.uint16

LSR = mybir.AluOpType.logical_shift_right
AND = mybir.AluOpType.bitwise_and
MULT = mybir.AluOpType.mult
ADD = mybir.AluOpType.add


def build_program(m_tiles=M // 128):
    nc = bacc.Bacc("TRN2", target_bir_lowering=False, debug=False, num_devices=N_CORES)

    Xd = nc.dram_tensor("x", [m_tiles, 128, K], BF16, kind="ExternalInput").ap()
    QWd = nc.dram_tensor("qw", [K, CS], I32, kind="ExternalInput").ap()
    Sd = nc.dram_tensor("s_bf", [NCH, NS], BF16, kind="ExternalInput").ap()
    ZSNd = nc.dram_tensor("zsn4", [128, NS], BF16, kind="ExternalInput").ap()
    XGd = nc.dram_tensor("xg4", [128, m_tiles * 128], BF16, kind="ExternalInput").ap()
    Bd = nc.dram_tensor("bias", [1, NS], F32, kind="ExternalInput").ap()
    Od = nc.dram_tensor("out", [m_tiles * 128, NS], F32, kind="ExternalOutput").ap()

    with tile.TileContext(nc) as tc:
        with (
            tc.tile_pool(name="wpool", bufs=1) as wpool,
            tc.tile_pool(name="meta", bufs=1) as meta,
            tc.tile_pool(name="qpool", bufs=4) as qpool,
            tc.tile_pool(name="qip", bufs=1) as qip,
            tc.tile_pool(name="bcast", bufs=3) as bcast,
            tc.tile_pool(name="xt", bufs=5) as xtp,
            tc.tile_pool(name="op", bufs=4) as outp,
            tc.tile_pool(name="ps", bufs=8, space="PSUM") as psp,
        ):
            # Resident dequantized weights [128k, chunk, n] = q * s
            W = wpool.tile([128, NCH, NS], BF16)
            bias_bc = meta.tile([128, NS], F32)
            zsn = meta.tile([128, NS], BF16)  # -z*s, replicated x4 row groups
            xg4 = meta.tile([128, m_tiles * 128], BF16)  # xg.T, replicated x4

            def emit_transpose(mt, pieces):
                """Plain DMA of the host-pretiled xT image: [p, g, m]."""
                xt = xtp.tile([128, NCH, 128], BF16, tag="xT", name=f"xT{mt}")
                kn = NCH // pieces
                for i in range(pieces):
                    nc.sync.dma_start(
                        xt[:, i * kn : (i + 1) * kn, :],
                        Xd[mt, :, i * kn * 128 : (i + 1) * kn * 128],
                    )
                return xt

            def emit_qwt(p, eng=None):
                qwt = qpool.tile([128, 2, CS], I32, tag="qwt", name="qwt")
                (eng or nc.sync).dma_start(
                    qwt[:],
                    QWd[p * 256 : (p + 1) * 256, :].rearrange(
                        "(two p) c -> p two c", p=128
                    ),
                )
                return qwt

            # Prestage phase-A m-tiles; m0 split fine so matmul g0 starts
            # early. qwt loads interleave between transposes on the sync ring
            # so the first dequant passes aren't starved by the x burst.
            xT = {0: emit_transpose(0, 4)}
            xT[1] = emit_transpose(1, 2)
            xT[2] = emit_transpose(2, 2)

            # Phase A PSUM groups: (mi, nt) -> psA[mi*2+nt], 512 cols each
            psA = [
                psp.tile([128, 512], F32, tag="pt", name=f"psA{j}")
                for j in range(2 * PHA_M)
            ]

            # HAM warm-up: dummy matmuls on xT0's first piece while the first
            # W chunk is still in flight -- the PE would idle here anyway, and
            # ~3.4us of sustained activity unthrottles the clock gate from
            # 1.2 to 2.4 GHz before the real accumulation chains begin.
            for _ in range(14):
                nc.tensor.matmul(
                    psA[0], xT[0][:, 0, :], xT[0][:, 0:4, :], start=True, stop=True
                )

            for p in range(NCH // 2):  # pass p covers chunks 2p, 2p+1
                qwt = emit_qwt(p, nc.scalar)
                # one scale broadcast DMA per pass: [128, chunk(2), NS] bf16
                szbc = bcast.tile([128, 2, NS], BF16, tag="szbc", name="szbc")
                if p < 2:  # split per chunk: finer-grained early pipelining
                    for j in range(2):
                        nc.scalar.dma_start(
                            szbc[:, j, :],
                            Sd[2 * p + j : 2 * p + j + 1, :][None].to_broadcast(
                                [128, 1, NS]
                            ),
                        )
                else:
                    nc.scalar.dma_start(
                        szbc[:],
                        Sd[2 * p : 2 * p + 2, :][None].to_broadcast([128, 2, NS]),
                    )
                # (q >> 4i) & 0x000F000F puts nibbles i, i+4 in the lo/hi
                # halfwords; int32 write at stride-4 offset i lands the uint16
                # view in exact logical column order.
                qint = qip.tile([128, 2, NS // 2], I32, tag="qint", name="qint")
                for i in range(4):
                    nc.vector.tensor_scalar(
                        qint[:, :, i::4], qwt[:], 4 * i, 0x000F000F, LSR, AND
                    )
                qint16 = qint.bitcast(U16)  # [128, 2, NS] logical order
                for j, g in ((0, 2 * p), (1, 2 * p + 1)):
                    wg = W[:, g, :]
                    nc.vector.tensor_tensor(wg, qint16[:, j, :], szbc[:, j, :], MULT)
                    # chunk-major phase-A matmuls: 8 groups x 512 cols
                    for mi in range(PHA_M):
                        for nt in range(2):
                            nc.tensor.matmul(
                                psA[mi * 2 + nt],
                                xT[mi][:, g, :],
                                W[:, g, nt * 512 : (nt + 1) * 512],
                                start=(g == 0),
                                stop=False,
                            )
                if p == 8:
                    nc.sync.dma_start(zsn[:], ZSNd)
                    nc.sync.dma_start(xg4[:], XGd)
                elif p == 10:
                    nc.sync.dma_start(bias_bc[:], Bd.to_broadcast([128, NS]))

            # zero-point corrections (K=32 matmuls, one per phase-A group)
            for nt in range(2):
                for mi in range(PHA_M):
                    nc.tensor.matmul(
                        psA[mi * 2 + nt],
                        xg4[0:32, mi * 128 : (mi + 1) * 128],
                        zsn[0:32, nt * 512 : (nt + 1) * 512],
                        start=False,
                        stop=True,
                    )

            # Phase A drains: bias-add n0/n1 into output tiles
            ot = {
                mi: outp.tile([128, NS], F32, tag="ot", name=f"ot{mi}")
                for mi in range(PHA_M)
            }
            for mi in range(PHA_M):
                for nt in range(2):
                    n0, nsz = N_TILES[nt]
                    nc.vector.tensor_tensor(
                        ot[mi][:, n0 : n0 + nsz],
                        psA[mi * 2 + nt][:, :nsz],
                        bias_bc[:, n0 : n0 + nsz],
                        ADD,
                    )

            def chain_mms(xt_tile, n0, nsz):
                pt = psp.tile([128, 512], F32, tag="pt", name="pt")
                for g in range(NCH):
                    nc.tensor.matmul(
                        pt[:, :nsz],
                        xt_tile[:, g, :],
                        W[:, g, n0 : n0 + nsz],
                        start=(g == 0),
                        stop=False,
                    )
                return pt

            def correction(pt, mt, n0, nsz, rg):
                # K=32 zero-point matmul; rg selects the PE row group so two
                # paired corrections run concurrently (base_partition derives
                # tile_position).
                nc.tensor.matmul(
                    pt[:, :nsz],
                    xg4[32 * rg : 32 * rg + 32, mt * 128 : (mt + 1) * 128],
                    zsn[32 * rg : 32 * rg + 32, n0 : n0 + nsz],
                    start=False,
                    stop=True,
                )

            def drain(pt, ot_tile, n0, nsz):
                nc.vector.tensor_tensor(
                    ot_tile[:, n0 : n0 + nsz], pt[:, :nsz], bias_bc[:, n0 : n0 + nsz], ADD
                )

            # Phase B: paired m-tiles so zero-point corrections pack 2-way on
            # the PE's 32-row groups. First finish n2 for the phase-A tiles.
            n0, nsz = N_TILES[2]
            pa = chain_mms(xT[0], n0, nsz)
            pb = chain_mms(xT[1], n0, nsz)
            correction(pa, 0, n0, nsz, 0)
            correction(pb, 1, n0, nsz, 1)
            drain(pa, ot[0], n0, nsz)
            drain(pb, ot[1], n0, nsz)
            nc.scalar.dma_start(Od[0:128, :], ot[0][:])
            nc.scalar.dma_start(Od[128:256, :], ot[1][:])
            pc = chain_mms(xT[2], n0, nsz)
            correction(pc, 2, n0, nsz, 0)
            drain(pc, ot[2], n0, nsz)
            nc.scalar.dma_start(Od[256:384, :], ot[2][:])

            for ma in range(PHA_M, m_tiles - 1, 2):
                mb = ma + 1
                xa = emit_transpose(ma, 1)
                xb = emit_transpose(mb, 1)
                oa = outp.tile([128, NS], F32, tag="ot", name="ot")
                ob = outp.tile([128, NS], F32, tag="ot", name="ot")
                for n0, nsz in N_TILES:
                    pa = chain_mms(xa, n0, nsz)
                    pb = chain_mms(xb, n0, nsz)
                    correction(pa, ma, n0, nsz, 0)
                    correction(pb, mb, n0, nsz, 1)
                    drain(pa, oa, n0, nsz)
                    drain(pb, ob, n0, nsz)
                nc.scalar.dma_start(Od[ma * 128 : (ma + 1) * 128, :], oa[:])
                nc.scalar.dma_start(Od[mb * 128 : (mb + 1) * 128, :], ob[:])

            # final (odd) m-tile: per-n-tile writeback overlaps the drains
            mt = m_tiles - 1
            xl = emit_transpose(mt, 1)
            ol = outp.tile([128, NS], F32, tag="ot", name="ot")
            for n0, nsz in N_TILES:
                pl = chain_mms(xl, n0, nsz)
                correction(pl, mt, n0, nsz, 0)
                drain(pl, ol, n0, nsz)
                nc.scalar.dma_start(
                    Od[mt * 128 : (mt + 1) * 128, n0 : n0 + nsz], ol[:, n0 : n0 + nsz]
                )

    nc.compile()
    return nc


def shard_inputs(x, qweight, scales, qzeros, bias, m_tiles=M // 128):
    """Host-side sharding + dtype prep (qzeros unpack, group sums, bf16 casts)."""
    # unpack qzeros [NCH, N//8] -> z_int [NCH, N] in logical column order
    shifts = np.array([0, 16, 4, 20, 8, 24, 12, 28], dtype=np.int32)  # 4*AWQ_ORDER
    z_int = ((qzeros[:, :, None] >> shifts[None, None, :]) & 0xF).reshape(NCH, N)
    s_bf = scales.astype(ml_dtypes.bfloat16)
    zsn = (-(z_int.astype(np.float32) * scales)).astype(ml_dtypes.bfloat16)
    xf = np.ascontiguousarray(x[: m_tiles * 128])
    # pre-tiled x image: xm[mt, p, g, m] = x[mt*128+m, g*128+p] flattened to
    # [mt, 128, K] -- every x load is then a plain contiguous DMA (the
    # DMA-transpose path serializes against all other DMA traffic on TRN2).
    xb = xf.astype(ml_dtypes.bfloat16).reshape(m_tiles, 128, NCH, G)
    xm = np.ascontiguousarray(xb.transpose(0, 3, 2, 1)).reshape(m_tiles, 128, K)
    xg = xf.reshape(m_tiles * 128, NCH, G).sum(-1)  # [M, 32] fp32 group sums
    xg4 = np.tile(
        np.ascontiguousarray(xg.T).astype(ml_dtypes.bfloat16), (4, 1)
    )  # [128, M]
    in_maps = []
    for c in range(N_CORES):
        nsl = slice(c * NS, (c + 1) * NS)
        in_maps.append(
            {
                "x": xm,
                "qw": np.ascontiguousarray(qweight[:, c * CS : (c + 1) * CS]),
                "s_bf": np.ascontiguousarray(s_bf[:, nsl]),
                "zsn4": np.tile(np.ascontiguousarray(zsn[:, nsl]), (4, 1)),
                "xg4": xg4,
                "bias": np.ascontiguousarray(bias[nsl]).reshape(1, NS),
            }
        )
    return in_maps


_CACHED_NC = None


def get_program():
    global _CACHED_NC
    if _CACHED_NC is None:
        _CACHED_NC = build_program()
    return _CACHED_NC


def kernel(x, qweight, scales, qzeros, bias):
    x = np.asarray(x, dtype=np.float32)
    qweight = np.asarray(qweight, dtype=np.int32)
    scales = np.asarray(scales, dtype=np.float32)
    qzeros = np.asarray(qzeros, dtype=np.int32)
    bias = np.asarray(bias, dtype=np.float32)
    nc = get_program()
    in_maps = shard_inputs(x, qweight, scales, qzeros, bias)
    res = bass_utils.run_bass_kernel_spmd(nc, in_maps, core_ids=list(range(N_CORES)))
    out = np.concatenate([res.results[c]["out"] for c in range(N_CORES)], axis=1)
    return out.astype(np.float32, copy=False)
